# revision 2
# baseline (speedup 1.0000x reference)
"""Trainium2 Bass kernel for nn_LINKX (GNN message passing + dense head).

Contract: kernel(**inputs) takes FULL unsharded inputs (numpy arrays keyed as
in setup_inputs()) and returns the FULL [N, OUT_C] float32 output.

Strategy (8 cores, graph-parallel by destination node):
  - Fold the whole dense prologue algebraically:
        h  = leaky(A @ T + x @ NW2 + c)          T  = edge_lin_weight @ (I+cat1)
        g  = leaky(h @ W0.T + b0)                NW2 = node_w @ (I+cat2)
        y  = leaky(g @ W1.T + b1)
    where A is the sparse [N,N] matrix with A[dst,src] += edge_weight, and
    W0/W1 are the host-computed modulated+row-normalized synthesis weights.
  - Shard dst nodes across 8 cores (12500 each). Per core, per 128-dst block:
    gather the needed T rows by src via gpsimd.dma_gather (SWDGE), build a
    [128 edge, 128 dst] selection matrix S (S[e,d] = w_e * (dst_e == d)) with
    one dual-op DVE tensor_scalar against an iota constant, and accumulate
        psum[h, d] += G_chunk[e, h]^T . S[e, d]
    on the tensor engine.  The x-part and the two synthesis matmuls chain in
    feature-major layout; leaky+bias fuse into single ACT ops (Lrelu).
  - Output per core is [64, 12544] feature-major; host transposes/concats.
"""

import math
import numpy as np

import concourse.bacc as bacc
import concourse.mybir as mybir
import concourse.tile as tile

F32 = mybir.dt.float32
I16 = mybir.dt.int16
SLOPE = 0.01
RANK = 10

# -------------------- problem constants (hardcoded) --------------------
N_NODES = 100000
N_EDGES = 1600000
IN_C = 128
H = 128
OUT_C = 64
N_CORES = 8


class Cfg:
    """Static plan shared by all cores: group sizes are maxes across cores."""

    def __init__(self, n_nodes, n_cores, tbl_rows, out_c=OUT_C,
                 sb_blocks=8, nch=4, g_bufs=12, s_bufs=8,
                 max_call_cols=8, single_packet=True):
        self.max_call_cols = max_call_cols
        self.single_packet = single_packet
        self.n_nodes = n_nodes
        self.n_cores = n_cores
        self.out_c = out_c
        self.pn = n_nodes // n_cores
        assert self.pn * n_cores == n_nodes
        self.nblk = math.ceil(self.pn / 128)
        self.pn_pad = self.nblk * 128
        self.tbl_rows = tbl_rows
        self.nch = nch
        self.chunk = math.ceil(tbl_rows / nch)
        assert self.chunk <= 32768
        self.sb_blocks = sb_blocks
        self.g_bufs = g_bufs
        self.s_bufs = s_bufs
        self.superblocks = [
            list(range(s, min(s + sb_blocks, self.nblk)))
            for s in range(0, self.nblk, sb_blocks)
        ]
        # filled by plan():
        self.ncols = None        # [nblk][nch] int
        self.col_of = None       # dict (b,c) -> first global column
        self.calls = None        # list of dicts
        self.totcols = None


def plan(cfg, counts):
    """counts: [ncores, nblk, nch] per-(core, block, srcchunk) edge counts.
    Bakes shared group sizes (max over cores) and the gather-call layout."""
    mx = counts.max(axis=0)  # [nblk, nch]
    cfg.ncols = np.ceil(mx / 128).astype(np.int64)  # [nblk, nch]
    col_of = {}
    calls = []
    cur = 0
    for si, sb in enumerate(cfg.superblocks):
        for c in range(cfg.nch):
            span_off = cur
            for b in sb:
                if cfg.ncols[b, c] > 0:
                    col_of[(b, c)] = cur
                    cur += int(cfg.ncols[b, c])
            # split the (superblock, chunk) span into calls of <= max_call_cols
            off = span_off
            while off < cur:
                n = min(cfg.max_call_cols, cur - off)
                calls.append(dict(si=si, chunk=c, col_off=off, ncols=n,
                                  ci=len(calls)))
                off += n
    cfg.col_of = col_of
    cfg.calls = calls
    cfg.totcols = cur
    # column -> call index lookup
    call_of_col = np.zeros(max(cur, 1), dtype=np.int64)
    for ci, call in enumerate(calls):
        call_of_col[call["col_off"]:call["col_off"] + call["ncols"]] = ci
    cfg.call_of_col = call_of_col
    return cfg


def host_prep_core(cfg, k, src, dst, w):
    """Per-core gather-stream arrays. src/dst/w are the FULL edge arrays."""
    pn = cfg.pn
    m = (dst >= k * pn) & (dst < (k + 1) * pn)
    s_k = src[m].astype(np.int64)
    d_k = (dst[m].astype(np.int64) - k * pn)
    w_k = w[m].astype(np.float32)
    b_k = d_k >> 7                       # block id
    dloc_k = (d_k & 127).astype(np.float32)
    c_k = s_k // cfg.chunk               # src chunk
    srel_k = (s_k % cfg.chunk).astype(np.int16)

    # stream group id in (superblock, chunk, block) order
    nblk, nch = cfg.nblk, cfg.nch
    gid_key = np.zeros(nblk * nch, dtype=np.int64)
    base_slot = np.zeros(nblk * nch, dtype=np.int64)
    order_i = 0
    for sb in cfg.superblocks:
        for c in range(nch):
            for b in sb:
                if (b, c) in cfg.col_of:
                    gid_key[b * nch + c] = order_i
                    base_slot[b * nch + c] = cfg.col_of[(b, c)] * 128
                    order_i += 1
    gid = gid_key[b_k * nch + c_k]
    order = np.argsort(gid, kind="stable")
    gid_s = gid[order]
    # rank within group
    grp_change = np.empty(len(gid_s), dtype=bool)
    if len(gid_s):
        grp_change[0] = True
        grp_change[1:] = gid_s[1:] != gid_s[:-1]
    grp_start = np.maximum.accumulate(np.where(grp_change, np.arange(len(gid_s)), 0))
    rank = np.arange(len(gid_s)) - grp_start
    slot = base_slot[(b_k * nch + c_k)[order]] + rank

    tot = cfg.totcols * 128
    idx_lin = np.zeros(tot, dtype=np.int16)
    dst_lin = np.full(tot, -1.0, dtype=np.float32)
    w_lin = np.zeros(tot, dtype=np.float32)
    idx_lin[slot] = srel_k[order]
    dst_lin[slot] = dloc_k[order]
    w_lin[slot] = w_k[order]

    idx2d = np.ascontiguousarray(np.tile(idx_lin.reshape(-1, 16).T, (8, 1)))
    dst2d = np.ascontiguousarray(dst_lin.reshape(-1, 128).T)
    w2d = np.ascontiguousarray(w_lin.reshape(-1, 128).T)
    return idx2d, dst2d, w2d


def host_weights(inputs):
    """Fold the dense algebra on host (float64 for the tiny mats)."""
    f8 = np.float64
    I = np.eye(H, dtype=f8)
    cat1 = np.asarray(inputs["cat1_w"], f8)
    cat2 = np.asarray(inputs["cat2_w"], f8)
    node_w = np.asarray(inputs["node_w"], f8)
    C1 = I + cat1
    C2 = I + cat2
    NW2 = node_w @ C2
    c = (np.asarray(inputs["edge_lin_bias"], f8) @ C1
         + np.asarray(inputs["cat1_b"], f8)
         + np.asarray(inputs["node_b"], f8) @ C2
         + np.asarray(inputs["cat2_b"], f8))
    # synthesis weights
    wvec = np.asarray(inputs["w"], f8)

    def synth(aff_w, aff_b, weight):
        c_out, c_in = weight.shape
        styles = wvec[0 if c_out == H else 1] @ np.asarray(aff_w, f8) + np.asarray(aff_b, f8)
        left = styles[: c_out * RANK].reshape(c_out, RANK)
        right = styles[c_out * RANK:].reshape(RANK, c_in)
        mod = (left @ right) / np.sqrt(np.float64(RANK))
        W = np.asarray(weight, f8) * (mod + 1.0)
        W = W / (np.linalg.norm(W, axis=1, keepdims=True) + 1e-8)
        return W

    W0 = synth(inputs["syn0_aff_w"], inputs["syn0_aff_b"], np.asarray(inputs["syn0_weight"], f8))
    W1 = synth(inputs["syn1_aff_w"], inputs["syn1_aff_b"], np.asarray(inputs["syn1_weight"], f8))

    # the big gather table: T = edge_lin_weight @ C1 (float32 matmul is fine)
    T = np.asarray(inputs["edge_lin_weight"], np.float32) @ C1.astype(np.float32)

    return dict(
        T=np.ascontiguousarray(T, np.float32),
        NW2=np.ascontiguousarray(NW2, np.float32),
        cvec=np.ascontiguousarray(c.reshape(H, 1), np.float32),
        W0T=np.ascontiguousarray(W0.T, np.float32),
        W1T=np.ascontiguousarray(W1.T, np.float32),
        b0=np.ascontiguousarray(np.asarray(inputs["syn0_bias"], f8).reshape(H, 1), np.float32),
        b1=np.ascontiguousarray(np.asarray(inputs["syn1_bias"], f8).reshape(OUT_C, 1), np.float32),
    )


def build_kernel_body(tc, cfg, outs, ins):
    """Trace the kernel into TileContext. outs/ins are dicts of DRAM APs."""
    nc = tc.nc
    out_c = cfg.out_c
    tbl, idxs, dstloc, wcol, xt = ins["tbl"], ins["idxs"], ins["dstloc"], ins["wcol"], ins["xt"]
    nw2, w0t, w1t = ins["nw2"], ins["w0t"], ins["w1t"]
    cvec, b0, b1, iota = ins["cvec"], ins["b0"], ins["b1"], ins["iota"]
    yout = outs["y"]

    eq = mybir.AluOpType.is_equal
    mul = mybir.AluOpType.mult
    LRELU = mybir.ActivationFunctionType.Lrelu

    calls_by_si = {}
    for call in cfg.calls:
        calls_by_si.setdefault(call["si"], []).append(call)

    with (
        tc.tile_pool(name="const", bufs=1) as cp,
        tc.tile_pool(name="gring", bufs=cfg.g_bufs) as gp,
        tc.tile_pool(name="spool", bufs=cfg.s_bufs) as sp,
        tc.tile_pool(name="hpool", bufs=4) as hp,
        tc.tile_pool(name="xtp", bufs=2) as xtp,
        tc.tile_pool(name="pacc", bufs=2, space="PSUM") as pacc,
        tc.tile_pool(name="p1", bufs=2, space="PSUM") as p1p,
        tc.tile_pool(name="p2", bufs=2, space="PSUM") as p2p,
    ):
        # ---- resident loads ----
        idx_sb = cp.tile([128, cfg.totcols * 8], I16)
        nc.sync.dma_start(idx_sb[:], idxs[:])
        dst_sb = cp.tile([128, cfg.totcols], F32)
        nc.sync.dma_start(dst_sb[:], dstloc[:])
        w_sb = cp.tile([128, cfg.totcols], F32)
        nc.sync.dma_start(w_sb[:], wcol[:])
        iota_sb = cp.tile([128, 128], F32)
        nc.sync.dma_start(iota_sb[:], iota[:])
        nw2_sb = cp.tile([H, H], F32)
        nc.sync.dma_start(nw2_sb[:], nw2[:])
        w0t_sb = cp.tile([H, H], F32)
        nc.sync.dma_start(w0t_sb[:], w0t[:])
        w1t_sb = cp.tile([H, out_c], F32)
        nc.sync.dma_start(w1t_sb[:], w1t[:])
        cvec_sb = cp.tile([H, 1], F32)
        nc.sync.dma_start(cvec_sb[:], cvec[:])
        b0_sb = cp.tile([H, 1], F32)
        nc.sync.dma_start(b0_sb[:], b0[:])
        b1_sb = cp.tile([out_c, 1], F32)
        nc.sync.dma_start(b1_sb[:], b1[:])
        y_sb = cp.tile([out_c, cfg.pn_pad], F32)

        for si, sb in enumerate(cfg.superblocks):
            sbn = len(sb)
            g_tiles = {}
            for call in calls_by_si.get(si, []):
                c = call["chunk"]
                ncols = call["ncols"]
                ni = ncols * 128
                gt = gp.tile([128, cfg.max_call_cols, 128], F32, tag="g")
                base = c * cfg.chunk
                rows = min(cfg.chunk, cfg.tbl_rows - base)
                nc.gpsimd.dma_gather(
                    gt[:, :ncols, :],
                    tbl[base:base + rows, :],
                    idx_sb[:, call["col_off"] * 8: (call["col_off"] + ncols) * 8],
                    ni, ni, H,
                    single_packet=cfg.single_packet,
                    queue_num=call["ci"] % 4,
                )
                g_tiles[call["ci"]] = (gt, call)

            xt_tile = xtp.tile([128, cfg.sb_blocks * 128], F32, tag="xt")
            nc.sync.dma_start(xt_tile[:, : sbn * 128],
                              xt[:, sb[0] * 128: sb[0] * 128 + sbn * 128])

            acc = pacc.tile([128, cfg.sb_blocks * 128], F32, tag="acc")
            # PSUM zero-regions are whole 2KB banks (4 x [128,128] windows):
            # exactly one start=True (first touch) and one stop=True (last
            # touch) per bank; all other matmuls accumulate with start=False.
            bank_started = [False] * ((sbn + 3) // 4)
            last_bank_window = {}
            for bi in range(sbn):
                last_bank_window[bi // 4] = bi
            for c in range(cfg.nch):
                for bi, b in enumerate(sb):
                    if (b, c) not in cfg.col_of:
                        continue
                    g0 = cfg.col_of[(b, c)]
                    for j in range(int(cfg.ncols[b, c])):
                        gcol = g0 + j
                        gt, call = g_tiles[int(cfg.call_of_col[gcol])]
                        jin = gcol - call["col_off"]
                        s_t = sp.tile([128, 128], F32, tag="s")
                        nc.vector.tensor_scalar(
                            s_t[:], iota_sb[:],
                            dst_sb[:, gcol:gcol + 1], w_sb[:, gcol:gcol + 1],
                            eq, mul,
                        )
                        nc.tensor.matmul(
                            acc[:, bi * 128:(bi + 1) * 128],
                            lhsT=gt[:, jin, :], rhs=s_t[:],
                            start=not bank_started[bi // 4], stop=False,
                        )
                        bank_started[bi // 4] = True
            for bi, b in enumerate(sb):
                nc.tensor.matmul(
                    acc[:, bi * 128:(bi + 1) * 128],
                    lhsT=nw2_sb[:], rhs=xt_tile[:, bi * 128:(bi + 1) * 128],
                    start=not bank_started[bi // 4],
                    stop=last_bank_window[bi // 4] == bi,
                )
                bank_started[bi // 4] = True
            for bi, b in enumerate(sb):
                h_t = hp.tile([128, 128], F32, tag="h")
                nc.scalar.activation(h_t[:], acc[:, bi * 128:(bi + 1) * 128],
                                     LRELU, bias=cvec_sb[:, 0:1], scale=1.0,
                                     alpha=SLOPE)
                ps1 = p1p.tile([H, 128], F32, tag="p1")
                nc.tensor.matmul(ps1[:], lhsT=w0t_sb[:], rhs=h_t[:],
                                 start=True, stop=True)
                g_t = hp.tile([128, 128], F32, tag="g2")
                nc.scalar.activation(g_t[:], ps1[:], LRELU,
                                     bias=b0_sb[:, 0:1], scale=1.0, alpha=SLOPE)
                ps2 = p2p.tile([out_c, 128], F32, tag="p2")
                nc.tensor.matmul(ps2[:], lhsT=w1t_sb[:], rhs=g_t[:],
                                 start=True, stop=True)
                nc.scalar.activation(y_sb[:, b * 128:(b + 1) * 128], ps2[:],
                                     LRELU, bias=b1_sb[:, 0:1], scale=1.0,
                                     alpha=SLOPE)

        nc.sync.dma_start(yout[:], y_sb[:])


def declare_tensors(nc, cfg):
    """DRAM tensor declarations; returns (ins, outs) dicts of APs."""
    d = nc.dram_tensor
    ins = dict(
        tbl=d("tbl", [cfg.tbl_rows, H], F32, kind="ExternalInput")[:, :],
        idxs=d("idxs", [128, cfg.totcols * 8], I16, kind="ExternalInput")[:, :],
        dstloc=d("dstloc", [128, cfg.totcols], F32, kind="ExternalInput")[:, :],
        wcol=d("wcol", [128, cfg.totcols], F32, kind="ExternalInput")[:, :],
        xt=d("xt", [H, cfg.pn_pad], F32, kind="ExternalInput")[:, :],
        nw2=d("nw2", [H, H], F32, kind="ExternalInput")[:, :],
        w0t=d("w0t", [H, H], F32, kind="ExternalInput")[:, :],
        w1t=d("w1t", [H, cfg.out_c], F32, kind="ExternalInput")[:, :],
        cvec=d("cvec", [H, 1], F32, kind="ExternalInput")[:, :],
        b0=d("b0", [H, 1], F32, kind="ExternalInput")[:, :],
        b1=d("b1", [cfg.out_c, 1], F32, kind="ExternalInput")[:, :],
        iota=d("iota", [128, 128], F32, kind="ExternalInput")[:, :],
    )
    outs = dict(y=d("y", [cfg.out_c, cfg.pn_pad], F32, kind="ExternalOutput")[:, :])
    return ins, outs


def make_iota():
    return np.ascontiguousarray(
        np.tile(np.arange(128, dtype=np.float32), (128, 1)))


def build_nc(cfg):
    nc = bacc.Bacc("TRN2", target_bir_lowering=False, debug=False,
                   num_devices=cfg.n_cores, num_swdge_queues=4)
    ins, outs = declare_tensors(nc, cfg)
    with tile.TileContext(nc) as tc:
        build_kernel_body(tc, cfg, outs, ins)
    nc.compile()
    return nc


def make_in_maps(cfg, inputs):
    """Full host prep: returns per-core input dicts + the plan cfg."""
    hw = host_weights(inputs)
    edge_index = np.asarray(inputs["edge_index"])
    src = edge_index[0].astype(np.int64)
    dst = edge_index[1].astype(np.int64)
    w = np.asarray(inputs["edge_weight"], np.float32)
    x = np.asarray(inputs["x"], np.float32)

    # per-(core, block, chunk) counts
    pn = cfg.pn
    core = dst // pn
    b = (dst % pn) >> 7
    c = src // cfg.chunk
    flat = (core * cfg.nblk + b) * cfg.nch + c
    counts = np.bincount(flat, minlength=cfg.n_cores * cfg.nblk * cfg.nch)
    counts = counts.reshape(cfg.n_cores, cfg.nblk, cfg.nch)
    plan(cfg, counts)

    iota = make_iota()
    in_maps = []
    for k in range(cfg.n_cores):
        idx2d, dst2d, w2d = host_prep_core(cfg, k, src, dst, w)
        xtk = np.zeros((H, cfg.pn_pad), np.float32)
        xtk[:, :pn] = x[k * pn:(k + 1) * pn].T
        in_maps.append(dict(
            tbl=hw["T"], idxs=idx2d, dstloc=dst2d, wcol=w2d,
            xt=np.ascontiguousarray(xtk),
            nw2=hw["NW2"], w0t=hw["W0T"], w1t=hw["W1T"],
            cvec=hw["cvec"], b0=hw["b0"], b1=hw["b1"], iota=iota,
        ))
    return in_maps


_CACHE = {}
LAST_RESULTS = None


def kernel(**inputs) -> np.ndarray:
    global LAST_RESULTS
    import os
    from concourse.bass_utils import run_bass_kernel_spmd

    cfg = Cfg(N_NODES, N_CORES, tbl_rows=N_NODES)
    in_maps = make_in_maps(cfg, inputs)

    key = ("nc", cfg.totcols)
    if key not in _CACHE:
        _CACHE[key] = build_nc(cfg)
    nc = _CACHE[key]

    trace = bool(int(os.environ.get("LINKX_TRACE", "0")))
    res = run_bass_kernel_spmd(nc, in_maps, core_ids=list(range(cfg.n_cores)),
                               trace=trace)
    LAST_RESULTS = res
    out = np.empty((N_NODES, OUT_C), np.float32)
    for k in range(cfg.n_cores):
        yk = res.results[k]["y"]
        out[k * cfg.pn:(k + 1) * cfg.pn] = yk[:, :cfg.pn].T
    return out



# revision 3
# speedup vs baseline: 1.0159x; 1.0159x over previous
"""Trainium2 Bass kernel for nn_LINKX (GNN message passing + dense head).

Contract: kernel(**inputs) takes FULL unsharded inputs (numpy arrays keyed as
in setup_inputs()) and returns the FULL [N, OUT_C] float32 output.

Strategy (8 cores, graph-parallel by destination node):
  - Fold the whole dense prologue algebraically:
        h  = leaky(A @ T + x @ NW2 + c)          T  = edge_lin_weight @ (I+cat1)
        g  = leaky(h @ W0.T + b0)                NW2 = node_w @ (I+cat2)
        y  = leaky(g @ W1.T + b1)
    where A is the sparse [N,N] matrix with A[dst,src] += edge_weight, and
    W0/W1 are the host-computed modulated+row-normalized synthesis weights.
  - Shard dst nodes across 8 cores (12500 each). Per core, per 128-dst block:
    gather the needed T rows by src via gpsimd.dma_gather (SWDGE), build a
    [128 edge, 128 dst] selection matrix S (S[e,d] = w_e * (dst_e == d)) with
    one dual-op DVE tensor_scalar against an iota constant, and accumulate
        psum[h, d] += G_chunk[e, h]^T . S[e, d]
    on the tensor engine.  The x-part and the two synthesis matmuls chain in
    feature-major layout; leaky+bias fuse into single ACT ops (Lrelu).
  - Output per core is [64, 12544] feature-major; host transposes/concats.
"""

import math
import numpy as np

import concourse.bacc as bacc
import concourse.mybir as mybir
import concourse.tile as tile

F32 = mybir.dt.float32
I16 = mybir.dt.int16
SLOPE = 0.01
RANK = 10

# -------------------- problem constants (hardcoded) --------------------
N_NODES = 100000
N_EDGES = 1600000
IN_C = 128
H = 128
OUT_C = 64
N_CORES = 8


class Cfg:
    """Static plan shared by all cores: group sizes are maxes across cores."""

    def __init__(self, n_nodes, n_cores, tbl_rows, out_c=OUT_C,
                 sb_blocks=8, nch=4, g_bufs=12, s_bufs=16,
                 max_call_cols=8, single_packet=True):
        self.max_call_cols = max_call_cols
        self.single_packet = single_packet
        self.n_nodes = n_nodes
        self.n_cores = n_cores
        self.out_c = out_c
        self.pn = n_nodes // n_cores
        assert self.pn * n_cores == n_nodes
        self.nblk = math.ceil(self.pn / 128)
        self.pn_pad = self.nblk * 128
        self.tbl_rows = tbl_rows
        self.nch = nch
        self.chunk = math.ceil(tbl_rows / nch)
        assert self.chunk <= 32768
        self.sb_blocks = sb_blocks
        self.g_bufs = g_bufs
        self.s_bufs = s_bufs
        self.superblocks = [
            list(range(s, min(s + sb_blocks, self.nblk)))
            for s in range(0, self.nblk, sb_blocks)
        ]
        # filled by plan():
        self.ncols = None        # [nblk][nch] int
        self.col_of = None       # dict (b,c) -> first global column
        self.calls = None        # list of dicts
        self.totcols = None


def plan(cfg, counts):
    """counts: [ncores, nblk, nch] per-(core, block, srcchunk) edge counts.
    Packs per-(block,chunk) budgets (max over cores, NOT rounded) back-to-back
    inside each (superblock, chunk) span; columns of 128 slots cut across
    block boundaries, each (column x block) overlap becoming one S-piece."""
    mx = counts.max(axis=0)  # [nblk, nch]
    cfg.budget = mx.astype(np.int64)
    cfg.slot_base = {}       # (b, c) -> global slot of block's budget start
    calls = []
    pieces_by_si = {}
    cur_col = 0
    scol = 0
    for si, sb in enumerate(cfg.superblocks):
        pieces = []
        for c in range(cfg.nch):
            span_col0 = cur_col
            off = 0
            block_ranges = []
            for b in sb:
                cfg.slot_base[(b, c)] = cur_col * 128 + off
                if mx[b, c] > 0:
                    block_ranges.append((b, off, off + int(mx[b, c])))
                off += int(mx[b, c])
            span_cols = (off + 127) // 128
            cur_col += span_cols
            # pieces: block budget range  x  128-slot columns
            for b, lo, hi in block_ranges:
                j0, j1 = lo // 128, (hi - 1) // 128
                for j in range(j0, j1 + 1):
                    plo = max(lo, j * 128) - j * 128
                    phi = min(hi, (j + 1) * 128) - j * 128
                    pieces.append(dict(col=span_col0 + j, bi=sb.index(b),
                                       lo=plo, hi=phi, scol=scol))
                    scol += 1
            # gather calls over this span's columns
            coff = span_col0
            while coff < span_col0 + span_cols:
                n = min(cfg.max_call_cols, span_col0 + span_cols - coff)
                calls.append(dict(si=si, chunk=c, col_off=coff, ncols=n,
                                  ci=len(calls)))
                coff += n
        pieces_by_si[si] = pieces
    cfg.calls = calls
    cfg.totcols = cur_col
    cfg.totscols = scol
    cfg.pieces_by_si = pieces_by_si
    call_of_col = np.zeros(max(cur_col, 1), dtype=np.int64)
    for ci, call in enumerate(calls):
        call_of_col[call["col_off"]:call["col_off"] + call["ncols"]] = ci
    cfg.call_of_col = call_of_col
    return cfg


def host_prep_core(cfg, k, src, dst, w):
    """Per-core gather + S-piece streams. src/dst/w are FULL edge arrays."""
    pn = cfg.pn
    m = (dst >= k * pn) & (dst < (k + 1) * pn)
    s_k = src[m].astype(np.int64)
    d_k = (dst[m].astype(np.int64) - k * pn)
    w_k = w[m].astype(np.float32)
    b_k = d_k >> 7
    dloc_k = (d_k & 127).astype(np.float32)
    c_k = s_k // cfg.chunk
    srel_k = (s_k % cfg.chunk).astype(np.int16)

    # rank within (b, c) group, then slot = slot_base[b,c] + rank
    nch = cfg.nch
    key = b_k * nch + c_k
    order = np.argsort(key, kind="stable")
    key_s = key[order]
    chg = np.empty(len(key_s), dtype=bool)
    if len(key_s):
        chg[0] = True
        chg[1:] = key_s[1:] != key_s[:-1]
    start = np.maximum.accumulate(np.where(chg, np.arange(len(key_s)), 0))
    rank = np.arange(len(key_s)) - start
    base_arr = np.zeros(cfg.nblk * nch, dtype=np.int64)
    for (b, c), sbase in cfg.slot_base.items():
        base_arr[b * nch + c] = sbase
    slot = base_arr[key_s] + rank

    tot = cfg.totcols * 128
    idx_lin = np.zeros(tot, dtype=np.int16)
    dst_lin = np.full(tot, -1.0, dtype=np.float32)
    w_lin = np.zeros(tot, dtype=np.float32)
    idx_lin[slot] = srel_k[order]
    dst_lin[slot] = dloc_k[order]
    w_lin[slot] = w_k[order]

    # per-piece S streams (mask outside the piece's slot window)
    stot = cfg.totscols * 128
    dstS = np.full(stot, -1.0, dtype=np.float32)
    wS = np.zeros(stot, dtype=np.float32)
    for si in range(len(cfg.superblocks)):
        for p in cfg.pieces_by_si[si]:
            colbase = p["col"] * 128
            sb0 = p["scol"] * 128
            dstS[sb0 + p["lo"]: sb0 + p["hi"]] = \
                dst_lin[colbase + p["lo"]: colbase + p["hi"]]
            wS[sb0 + p["lo"]: sb0 + p["hi"]] = \
                w_lin[colbase + p["lo"]: colbase + p["hi"]]

    idx2d = np.ascontiguousarray(np.tile(idx_lin.reshape(-1, 16).T, (8, 1)))
    dst2d = np.ascontiguousarray(dstS.reshape(-1, 128).T)
    w2d = np.ascontiguousarray(wS.reshape(-1, 128).T)
    return idx2d, dst2d, w2d


def host_weights(inputs):
    """Fold the dense algebra on host (float64 for the tiny mats)."""
    f8 = np.float64
    I = np.eye(H, dtype=f8)
    cat1 = np.asarray(inputs["cat1_w"], f8)
    cat2 = np.asarray(inputs["cat2_w"], f8)
    node_w = np.asarray(inputs["node_w"], f8)
    C1 = I + cat1
    C2 = I + cat2
    NW2 = node_w @ C2
    c = (np.asarray(inputs["edge_lin_bias"], f8) @ C1
         + np.asarray(inputs["cat1_b"], f8)
         + np.asarray(inputs["node_b"], f8) @ C2
         + np.asarray(inputs["cat2_b"], f8))
    # synthesis weights
    wvec = np.asarray(inputs["w"], f8)

    def synth(aff_w, aff_b, weight):
        c_out, c_in = weight.shape
        styles = wvec[0 if c_out == H else 1] @ np.asarray(aff_w, f8) + np.asarray(aff_b, f8)
        left = styles[: c_out * RANK].reshape(c_out, RANK)
        right = styles[c_out * RANK:].reshape(RANK, c_in)
        mod = (left @ right) / np.sqrt(np.float64(RANK))
        W = np.asarray(weight, f8) * (mod + 1.0)
        W = W / (np.linalg.norm(W, axis=1, keepdims=True) + 1e-8)
        return W

    W0 = synth(inputs["syn0_aff_w"], inputs["syn0_aff_b"], np.asarray(inputs["syn0_weight"], f8))
    W1 = synth(inputs["syn1_aff_w"], inputs["syn1_aff_b"], np.asarray(inputs["syn1_weight"], f8))

    # the big gather table: T = edge_lin_weight @ C1 (float32 matmul is fine)
    T = np.asarray(inputs["edge_lin_weight"], np.float32) @ C1.astype(np.float32)

    return dict(
        T=np.ascontiguousarray(T, np.float32),
        NW2=np.ascontiguousarray(NW2, np.float32),
        cvec=np.ascontiguousarray(c.reshape(H, 1), np.float32),
        W0T=np.ascontiguousarray(W0.T, np.float32),
        W1T=np.ascontiguousarray(W1.T, np.float32),
        b0=np.ascontiguousarray(np.asarray(inputs["syn0_bias"], f8).reshape(H, 1), np.float32),
        b1=np.ascontiguousarray(np.asarray(inputs["syn1_bias"], f8).reshape(OUT_C, 1), np.float32),
    )


def build_kernel_body(tc, cfg, outs, ins):
    """Trace the kernel into TileContext. outs/ins are dicts of DRAM APs."""
    nc = tc.nc
    out_c = cfg.out_c
    tbl, idxs, dstloc, wcol, xt = ins["tbl"], ins["idxs"], ins["dstloc"], ins["wcol"], ins["xt"]
    nw2, w0t, w1t = ins["nw2"], ins["w0t"], ins["w1t"]
    cvec, b0, b1, iota = ins["cvec"], ins["b0"], ins["b1"], ins["iota"]
    yout = outs["y"]

    eq = mybir.AluOpType.is_equal
    mul = mybir.AluOpType.mult
    LRELU = mybir.ActivationFunctionType.Lrelu

    calls_by_si = {}
    for call in cfg.calls:
        calls_by_si.setdefault(call["si"], []).append(call)

    with (
        tc.tile_pool(name="const", bufs=1) as cp,
        tc.tile_pool(name="gring", bufs=cfg.g_bufs) as gp,
        tc.tile_pool(name="spool", bufs=cfg.s_bufs) as sp,
        tc.tile_pool(name="hpool", bufs=4) as hp,
        tc.tile_pool(name="xtp", bufs=2) as xtp,
        tc.tile_pool(name="pacc", bufs=2, space="PSUM") as pacc,
        tc.tile_pool(name="p1", bufs=2, space="PSUM") as p1p,
        tc.tile_pool(name="p2", bufs=2, space="PSUM") as p2p,
    ):
        # ---- resident loads ----
        idx_sb = cp.tile([128, cfg.totcols * 8], I16)
        nc.sync.dma_start(idx_sb[:], idxs[:])
        dst_sb = cp.tile([128, cfg.totscols], F32)
        nc.sync.dma_start(dst_sb[:], dstloc[:])
        w_sb = cp.tile([128, cfg.totscols], F32)
        nc.sync.dma_start(w_sb[:], wcol[:])
        iota_sb = cp.tile([128, 128], F32)
        nc.sync.dma_start(iota_sb[:], iota[:])
        nw2_sb = cp.tile([H, H], F32)
        nc.sync.dma_start(nw2_sb[:], nw2[:])
        w0t_sb = cp.tile([H, H], F32)
        nc.sync.dma_start(w0t_sb[:], w0t[:])
        w1t_sb = cp.tile([H, out_c], F32)
        nc.sync.dma_start(w1t_sb[:], w1t[:])
        cvec_sb = cp.tile([H, 1], F32)
        nc.sync.dma_start(cvec_sb[:], cvec[:])
        b0_sb = cp.tile([H, 1], F32)
        nc.sync.dma_start(b0_sb[:], b0[:])
        b1_sb = cp.tile([out_c, 1], F32)
        nc.sync.dma_start(b1_sb[:], b1[:])
        y_sb = cp.tile([out_c, cfg.pn_pad], F32)

        for si, sb in enumerate(cfg.superblocks):
            sbn = len(sb)
            g_tiles = {}
            for call in calls_by_si.get(si, []):
                c = call["chunk"]
                ncols = call["ncols"]
                ni = ncols * 128
                gt = gp.tile([128, cfg.max_call_cols, 128], F32, tag="g")
                base = c * cfg.chunk
                rows = min(cfg.chunk, cfg.tbl_rows - base)
                nc.gpsimd.dma_gather(
                    gt[:, :ncols, :],
                    tbl[base:base + rows, :],
                    idx_sb[:, call["col_off"] * 8: (call["col_off"] + ncols) * 8],
                    ni, ni, H,
                    single_packet=cfg.single_packet,
                    queue_num=call["ci"] % 3,
                )
                g_tiles[call["ci"]] = (gt, call)

            xt_tile = xtp.tile([128, cfg.sb_blocks * 128], F32, tag="xt")
            nc.sync.dma_start(xt_tile[:, : sbn * 128],
                              xt[:, sb[0] * 128: sb[0] * 128 + sbn * 128])

            acc = pacc.tile([128, cfg.sb_blocks * 128], F32, tag="acc")
            # PSUM zero-regions are whole 2KB banks (4 x [128,128] windows):
            # exactly one start=True (first touch) and one stop=True (last
            # touch) per bank; all other matmuls accumulate with start=False.
            bank_started = [False] * ((sbn + 3) // 4)
            last_bank_window = {}
            for bi in range(sbn):
                last_bank_window[bi // 4] = bi
            for pi, p in enumerate(cfg.pieces_by_si[si]):
                gcol = p["col"]
                bi = p["bi"]
                scol = p["scol"]
                gt, call = g_tiles[int(cfg.call_of_col[gcol])]
                jin = gcol - call["col_off"]
                s_t = sp.tile([128, 128], F32, tag="s")
                nc.vector.tensor_scalar(
                    s_t[:], iota_sb[:],
                    dst_sb[:, scol:scol + 1], w_sb[:, scol:scol + 1],
                    eq, mul,
                )
                nc.tensor.matmul(
                    acc[:, bi * 128:(bi + 1) * 128],
                    lhsT=gt[:, jin, :], rhs=s_t[:],
                    start=not bank_started[bi // 4], stop=False,
                )
                bank_started[bi // 4] = True
            for bi, b in enumerate(sb):
                nc.tensor.matmul(
                    acc[:, bi * 128:(bi + 1) * 128],
                    lhsT=nw2_sb[:], rhs=xt_tile[:, bi * 128:(bi + 1) * 128],
                    start=not bank_started[bi // 4],
                    stop=last_bank_window[bi // 4] == bi,
                )
                bank_started[bi // 4] = True
            for bi, b in enumerate(sb):
                h_t = hp.tile([128, 128], F32, tag="h")
                nc.scalar.activation(h_t[:], acc[:, bi * 128:(bi + 1) * 128],
                                     LRELU, bias=cvec_sb[:, 0:1], scale=1.0,
                                     alpha=SLOPE)
                ps1 = p1p.tile([H, 128], F32, tag="p1")
                nc.tensor.matmul(ps1[:], lhsT=w0t_sb[:], rhs=h_t[:],
                                 start=True, stop=True)
                g_t = hp.tile([128, 128], F32, tag="g2")
                nc.scalar.activation(g_t[:], ps1[:], LRELU,
                                     bias=b0_sb[:, 0:1], scale=1.0, alpha=SLOPE)
                ps2 = p2p.tile([out_c, 128], F32, tag="p2")
                nc.tensor.matmul(ps2[:], lhsT=w1t_sb[:], rhs=g_t[:],
                                 start=True, stop=True)
                nc.scalar.activation(y_sb[:, b * 128:(b + 1) * 128], ps2[:],
                                     LRELU, bias=b1_sb[:, 0:1], scale=1.0,
                                     alpha=SLOPE)

        nc.sync.dma_start(yout[:], y_sb[:])


def declare_tensors(nc, cfg):
    """DRAM tensor declarations; returns (ins, outs) dicts of APs."""
    d = nc.dram_tensor
    ins = dict(
        tbl=d("tbl", [cfg.tbl_rows, H], F32, kind="ExternalInput")[:, :],
        idxs=d("idxs", [128, cfg.totcols * 8], I16, kind="ExternalInput")[:, :],
        dstloc=d("dstloc", [128, cfg.totscols], F32, kind="ExternalInput")[:, :],
        wcol=d("wcol", [128, cfg.totscols], F32, kind="ExternalInput")[:, :],
        xt=d("xt", [H, cfg.pn_pad], F32, kind="ExternalInput")[:, :],
        nw2=d("nw2", [H, H], F32, kind="ExternalInput")[:, :],
        w0t=d("w0t", [H, H], F32, kind="ExternalInput")[:, :],
        w1t=d("w1t", [H, cfg.out_c], F32, kind="ExternalInput")[:, :],
        cvec=d("cvec", [H, 1], F32, kind="ExternalInput")[:, :],
        b0=d("b0", [H, 1], F32, kind="ExternalInput")[:, :],
        b1=d("b1", [cfg.out_c, 1], F32, kind="ExternalInput")[:, :],
        iota=d("iota", [128, 128], F32, kind="ExternalInput")[:, :],
    )
    outs = dict(y=d("y", [cfg.out_c, cfg.pn_pad], F32, kind="ExternalOutput")[:, :])
    return ins, outs


def make_iota():
    return np.ascontiguousarray(
        np.tile(np.arange(128, dtype=np.float32), (128, 1)))


def build_nc(cfg):
    nc = bacc.Bacc("TRN2", target_bir_lowering=False, debug=False,
                   num_devices=cfg.n_cores, num_swdge_queues=3)
    ins, outs = declare_tensors(nc, cfg)
    with tile.TileContext(nc) as tc:
        build_kernel_body(tc, cfg, outs, ins)
    nc.compile()
    return nc


def make_in_maps(cfg, inputs):
    """Full host prep: returns per-core input dicts + the plan cfg."""
    hw = host_weights(inputs)
    edge_index = np.asarray(inputs["edge_index"])
    src = edge_index[0].astype(np.int64)
    dst = edge_index[1].astype(np.int64)
    w = np.asarray(inputs["edge_weight"], np.float32)
    x = np.asarray(inputs["x"], np.float32)

    # per-(core, block, chunk) counts
    pn = cfg.pn
    core = dst // pn
    b = (dst % pn) >> 7
    c = src // cfg.chunk
    flat = (core * cfg.nblk + b) * cfg.nch + c
    counts = np.bincount(flat, minlength=cfg.n_cores * cfg.nblk * cfg.nch)
    counts = counts.reshape(cfg.n_cores, cfg.nblk, cfg.nch)
    plan(cfg, counts)

    iota = make_iota()
    in_maps = []
    for k in range(cfg.n_cores):
        idx2d, dst2d, w2d = host_prep_core(cfg, k, src, dst, w)
        xtk = np.zeros((H, cfg.pn_pad), np.float32)
        xtk[:, :pn] = x[k * pn:(k + 1) * pn].T
        in_maps.append(dict(
            tbl=hw["T"], idxs=idx2d, dstloc=dst2d, wcol=w2d,
            xt=np.ascontiguousarray(xtk),
            nw2=hw["NW2"], w0t=hw["W0T"], w1t=hw["W1T"],
            cvec=hw["cvec"], b0=hw["b0"], b1=hw["b1"], iota=iota,
        ))
    return in_maps


_CACHE = {}
LAST_RESULTS = None


def kernel(**inputs) -> np.ndarray:
    global LAST_RESULTS
    import os
    from concourse.bass_utils import run_bass_kernel_spmd

    cfg = Cfg(N_NODES, N_CORES, tbl_rows=N_NODES)
    in_maps = make_in_maps(cfg, inputs)

    key = ("nc", cfg.totcols, cfg.totscols)
    if key not in _CACHE:
        _CACHE[key] = build_nc(cfg)
    nc = _CACHE[key]

    trace = bool(int(os.environ.get("LINKX_TRACE", "0")))
    res = run_bass_kernel_spmd(nc, in_maps, core_ids=list(range(cfg.n_cores)),
                               trace=trace)
    LAST_RESULTS = res
    out = np.empty((N_NODES, OUT_C), np.float32)
    for k in range(cfg.n_cores):
        yk = res.results[k]["y"]
        out[k * cfg.pn:(k + 1) * cfg.pn] = yk[:, :cfg.pn].T
    return out



# revision 4
# speedup vs baseline: 1.3838x; 1.3621x over previous
"""Trainium2 Bass kernel for nn_LINKX (GNN message passing + dense head).

Contract: kernel(**inputs) takes FULL unsharded inputs (numpy arrays keyed as
in setup_inputs()) and returns the FULL [N, OUT_C] float32 output.

Strategy (8 cores, graph-parallel by destination node):
  - Fold the whole dense prologue algebraically:
        h  = leaky(A @ T + x @ NW2 + c)          T  = edge_lin_weight @ (I+cat1)
        g  = leaky(h @ W0.T + b0)                NW2 = node_w @ (I+cat2)
        y  = leaky(g @ W1.T + b1)
    where A is the sparse [N,N] matrix with A[dst,src] += edge_weight, and
    W0/W1 are the host-computed modulated+row-normalized synthesis weights.
  - Shard dst nodes across 8 cores (12500 each). Per core, per 128-dst block:
    gather the needed T rows by src via gpsimd.dma_gather (SWDGE), build a
    [128 edge, 128 dst] selection matrix S (S[e,d] = w_e * (dst_e == d)) with
    one dual-op DVE tensor_scalar against an iota constant, and accumulate
        psum[h, d] += G_chunk[e, h]^T . S[e, d]
    on the tensor engine.  The x-part and the two synthesis matmuls chain in
    feature-major layout; leaky+bias fuse into single ACT ops (Lrelu).
  - Output per core is [64, 12544] feature-major; host transposes/concats.
"""

import math
import numpy as np

import concourse.bacc as bacc
import concourse.mybir as mybir
import concourse.tile as tile

F32 = mybir.dt.float32
I16 = mybir.dt.int16
SLOPE = 0.01
RANK = 10

# -------------------- problem constants (hardcoded) --------------------
N_NODES = 100000
N_EDGES = 1600000
IN_C = 128
H = 128
OUT_C = 64
N_CORES = 8


class Cfg:
    """Static plan shared by all cores: group sizes are maxes across cores."""

    def __init__(self, n_nodes, n_cores, tbl_rows, out_c=OUT_C,
                 sb_blocks=8, nch=4, g_bufs=12, s_bufs=16,
                 max_call_cols=8, single_packet=True):
        self.max_call_cols = max_call_cols
        self.single_packet = single_packet
        self.n_nodes = n_nodes
        self.n_cores = n_cores
        self.out_c = out_c
        self.pn = n_nodes // n_cores
        assert self.pn * n_cores == n_nodes
        self.nblk = math.ceil(self.pn / 128)
        self.pn_pad = self.nblk * 128
        self.tbl_rows = tbl_rows
        self.nch = nch
        self.chunk = math.ceil(tbl_rows / nch)
        assert self.chunk <= 32768
        self.sb_blocks = sb_blocks
        self.g_bufs = g_bufs
        self.s_bufs = s_bufs
        self.superblocks = [
            list(range(s, min(s + sb_blocks, self.nblk)))
            for s in range(0, self.nblk, sb_blocks)
        ]
        # filled by plan():
        self.ncols = None        # [nblk][nch] int
        self.col_of = None       # dict (b,c) -> first global column
        self.calls = None        # list of dicts
        self.totcols = None


def plan(cfg, counts):
    """counts: [ncores, nblk, nch] per-(core, block, srcchunk) edge counts.
    Packs per-(block,chunk) budgets (max over cores, NOT rounded) back-to-back
    inside each (superblock, chunk) span; columns of 128 slots cut across
    block boundaries, each (column x block) overlap becoming one S-piece."""
    mx = counts.max(axis=0)  # [nblk, nch]
    cfg.budget = mx.astype(np.int64)
    cfg.slot_base = {}       # (b, c) -> global slot of block's budget start
    calls = []
    pieces_by_si = {}
    cur_col = 0
    scol = 0
    for si, sb in enumerate(cfg.superblocks):
        pieces = []
        for c in range(cfg.nch):
            span_col0 = cur_col
            off = 0
            block_ranges = []
            for b in sb:
                cfg.slot_base[(b, c)] = cur_col * 128 + off
                if mx[b, c] > 0:
                    block_ranges.append((b, off, off + int(mx[b, c])))
                off += int(mx[b, c])
            span_cols = (off + 127) // 128
            cur_col += span_cols
            # pieces: block budget range x 128-slot columns; adjacent
            # same-bank segments of one column merge into a 256-wide piece
            # (second block's dst values get +128 host-side).
            col_segs = {}
            for b, lo, hi in block_ranges:
                j0, j1 = lo // 128, (hi - 1) // 128
                for j in range(j0, j1 + 1):
                    plo = max(lo, j * 128) - j * 128
                    phi = min(hi, (j + 1) * 128) - j * 128
                    col_segs.setdefault(j, []).append((sb.index(b), plo, phi))
            for j in sorted(col_segs):
                segs = col_segs[j]
                i = 0
                while i < len(segs):
                    bi0, lo0, hi0 = segs[i]
                    if (i + 1 < len(segs)
                            and segs[i + 1][0] == bi0 + 1
                            and (bi0 + 1) // 4 == bi0 // 4):
                        bi1, lo1, hi1 = segs[i + 1]
                        pieces.append(dict(col=span_col0 + j, bi=bi0,
                                           segs=[(lo0, hi0, 0),
                                                 (lo1, hi1, 128)],
                                           width=256, scol=scol))
                        i += 2
                    else:
                        pieces.append(dict(col=span_col0 + j, bi=bi0,
                                           segs=[(lo0, hi0, 0)],
                                           width=128, scol=scol))
                        i += 1
                    scol += 1
            # gather calls over this span's columns
            coff = span_col0
            while coff < span_col0 + span_cols:
                n = min(cfg.max_call_cols, span_col0 + span_cols - coff)
                calls.append(dict(si=si, chunk=c, col_off=coff, ncols=n,
                                  ci=len(calls)))
                coff += n
        pieces_by_si[si] = pieces
    cfg.calls = calls
    cfg.totcols = cur_col
    cfg.totscols = scol
    cfg.pieces_by_si = pieces_by_si
    call_of_col = np.zeros(max(cur_col, 1), dtype=np.int64)
    for ci, call in enumerate(calls):
        call_of_col[call["col_off"]:call["col_off"] + call["ncols"]] = ci
    cfg.call_of_col = call_of_col
    return cfg


def host_prep_core(cfg, k, src, dst, w):
    """Per-core gather + S-piece streams. src/dst/w are FULL edge arrays."""
    pn = cfg.pn
    m = (dst >= k * pn) & (dst < (k + 1) * pn)
    s_k = src[m].astype(np.int64)
    d_k = (dst[m].astype(np.int64) - k * pn)
    w_k = w[m].astype(np.float32)
    b_k = d_k >> 7
    dloc_k = (d_k & 127).astype(np.float32)
    c_k = s_k // cfg.chunk
    srel_k = (s_k % cfg.chunk).astype(np.int16)

    # rank within (b, c) group, then slot = slot_base[b,c] + rank
    nch = cfg.nch
    key = b_k * nch + c_k
    order = np.argsort(key, kind="stable")
    key_s = key[order]
    chg = np.empty(len(key_s), dtype=bool)
    if len(key_s):
        chg[0] = True
        chg[1:] = key_s[1:] != key_s[:-1]
    start = np.maximum.accumulate(np.where(chg, np.arange(len(key_s)), 0))
    rank = np.arange(len(key_s)) - start
    base_arr = np.zeros(cfg.nblk * nch, dtype=np.int64)
    for (b, c), sbase in cfg.slot_base.items():
        base_arr[b * nch + c] = sbase
    slot = base_arr[key_s] + rank

    tot = cfg.totcols * 128
    idx_lin = np.zeros(tot, dtype=np.int16)
    dst_lin = np.full(tot, -1.0, dtype=np.float32)
    w_lin = np.zeros(tot, dtype=np.float32)
    idx_lin[slot] = srel_k[order]
    dst_lin[slot] = dloc_k[order]
    w_lin[slot] = w_k[order]

    # per-piece S streams (mask outside the piece's slot window)
    stot = cfg.totscols * 128
    dstS = np.full(stot, -1.0, dtype=np.float32)
    wS = np.zeros(stot, dtype=np.float32)
    for si in range(len(cfg.superblocks)):
        for p in cfg.pieces_by_si[si]:
            colbase = p["col"] * 128
            sb0 = p["scol"] * 128
            for lo, hi, off in p["segs"]:
                seg = dst_lin[colbase + lo: colbase + hi]
                dstS[sb0 + lo: sb0 + hi] = np.where(seg >= 0, seg + off, -1.0)
                wS[sb0 + lo: sb0 + hi] = w_lin[colbase + lo: colbase + hi]

    idx2d = np.ascontiguousarray(np.tile(idx_lin.reshape(-1, 16).T, (8, 1)))
    dst2d = np.ascontiguousarray(dstS.reshape(-1, 128).T)
    w2d = np.ascontiguousarray(wS.reshape(-1, 128).T)
    return idx2d, dst2d, w2d


def host_weights(inputs):
    """Fold the dense algebra on host (float64 for the tiny mats)."""
    f8 = np.float64
    I = np.eye(H, dtype=f8)
    cat1 = np.asarray(inputs["cat1_w"], f8)
    cat2 = np.asarray(inputs["cat2_w"], f8)
    node_w = np.asarray(inputs["node_w"], f8)
    C1 = I + cat1
    C2 = I + cat2
    NW2 = node_w @ C2
    c = (np.asarray(inputs["edge_lin_bias"], f8) @ C1
         + np.asarray(inputs["cat1_b"], f8)
         + np.asarray(inputs["node_b"], f8) @ C2
         + np.asarray(inputs["cat2_b"], f8))
    # synthesis weights
    wvec = np.asarray(inputs["w"], f8)

    def synth(aff_w, aff_b, weight):
        c_out, c_in = weight.shape
        styles = wvec[0 if c_out == H else 1] @ np.asarray(aff_w, f8) + np.asarray(aff_b, f8)
        left = styles[: c_out * RANK].reshape(c_out, RANK)
        right = styles[c_out * RANK:].reshape(RANK, c_in)
        mod = (left @ right) / np.sqrt(np.float64(RANK))
        W = np.asarray(weight, f8) * (mod + 1.0)
        W = W / (np.linalg.norm(W, axis=1, keepdims=True) + 1e-8)
        return W

    W0 = synth(inputs["syn0_aff_w"], inputs["syn0_aff_b"], np.asarray(inputs["syn0_weight"], f8))
    W1 = synth(inputs["syn1_aff_w"], inputs["syn1_aff_b"], np.asarray(inputs["syn1_weight"], f8))

    # the big gather table: T = edge_lin_weight @ C1 (float32 matmul is fine)
    T = np.asarray(inputs["edge_lin_weight"], np.float32) @ C1.astype(np.float32)

    return dict(
        T=np.ascontiguousarray(T, np.float32),
        NW2=np.ascontiguousarray(NW2, np.float32),
        cvec=np.ascontiguousarray(c.reshape(H, 1), np.float32),
        W0T=np.ascontiguousarray(W0.T, np.float32),
        W1T=np.ascontiguousarray(W1.T, np.float32),
        b0=np.ascontiguousarray(np.asarray(inputs["syn0_bias"], f8).reshape(H, 1), np.float32),
        b1=np.ascontiguousarray(np.asarray(inputs["syn1_bias"], f8).reshape(OUT_C, 1), np.float32),
    )


def build_kernel_body(tc, cfg, outs, ins):
    """Trace the kernel into TileContext. outs/ins are dicts of DRAM APs."""
    nc = tc.nc
    out_c = cfg.out_c
    tbl, idxs, dstloc, wcol, xt = ins["tbl"], ins["idxs"], ins["dstloc"], ins["wcol"], ins["xt"]
    nw2, w0t, w1t = ins["nw2"], ins["w0t"], ins["w1t"]
    cvec, b0, b1, iota = ins["cvec"], ins["b0"], ins["b1"], ins["iota"]
    yout = outs["y"]

    eq = mybir.AluOpType.is_equal
    mul = mybir.AluOpType.mult
    LRELU = mybir.ActivationFunctionType.Lrelu

    calls_by_si = {}
    for call in cfg.calls:
        calls_by_si.setdefault(call["si"], []).append(call)

    with (
        tc.tile_pool(name="const", bufs=1) as cp,
        tc.tile_pool(name="gring", bufs=cfg.g_bufs) as gp,
        tc.tile_pool(name="spool", bufs=cfg.s_bufs) as sp,
        tc.tile_pool(name="hpool", bufs=4) as hp,
        tc.tile_pool(name="xtp", bufs=2) as xtp,
        tc.tile_pool(name="pacc", bufs=2, space="PSUM") as pacc,
        tc.tile_pool(name="p1", bufs=2, space="PSUM") as p1p,
        tc.tile_pool(name="p2", bufs=2, space="PSUM") as p2p,
    ):
        # ---- resident loads ----
        idx_sb = cp.tile([128, cfg.totcols * 8], I16)
        nc.sync.dma_start(idx_sb[:], idxs[:])
        dst_sb = cp.tile([128, cfg.totscols], F32)
        nc.sync.dma_start(dst_sb[:], dstloc[:])
        w_sb = cp.tile([128, cfg.totscols], F32)
        nc.sync.dma_start(w_sb[:], wcol[:])
        iota_sb = cp.tile([128, 256], F32)
        nc.sync.dma_start(iota_sb[:], iota[:])
        nw2_sb = cp.tile([H, H], F32)
        nc.sync.dma_start(nw2_sb[:], nw2[:])
        w0t_sb = cp.tile([H, H], F32)
        nc.sync.dma_start(w0t_sb[:], w0t[:])
        w1t_sb = cp.tile([H, out_c], F32)
        nc.sync.dma_start(w1t_sb[:], w1t[:])
        cvec_sb = cp.tile([H, 1], F32)
        nc.sync.dma_start(cvec_sb[:], cvec[:])
        b0_sb = cp.tile([H, 1], F32)
        nc.sync.dma_start(b0_sb[:], b0[:])
        b1_sb = cp.tile([out_c, 1], F32)
        nc.sync.dma_start(b1_sb[:], b1[:])
        y_sb = cp.tile([out_c, cfg.pn_pad], F32)

        for si, sb in enumerate(cfg.superblocks):
            sbn = len(sb)
            g_tiles = {}
            for call in calls_by_si.get(si, []):
                c = call["chunk"]
                ncols = call["ncols"]
                ni = ncols * 128
                gt = gp.tile([128, cfg.max_call_cols, 128], F32, tag="g")
                base = c * cfg.chunk
                rows = min(cfg.chunk, cfg.tbl_rows - base)
                nc.gpsimd.dma_gather(
                    gt[:, :ncols, :],
                    tbl[base:base + rows, :],
                    idx_sb[:, call["col_off"] * 8: (call["col_off"] + ncols) * 8],
                    ni, ni, H,
                    single_packet=cfg.single_packet,
                    queue_num=call["ci"] % 3,
                )
                g_tiles[call["ci"]] = (gt, call)

            xt_tile = xtp.tile([128, cfg.sb_blocks * 128], F32, tag="xt")
            nc.sync.dma_start(xt_tile[:, : sbn * 128],
                              xt[:, sb[0] * 128: sb[0] * 128 + sbn * 128])

            acc = pacc.tile([128, cfg.sb_blocks * 128], F32, tag="acc")
            # PSUM zero-regions are whole 2KB banks (4 x [128,128] windows):
            # exactly one start=True (first touch) and one stop=True (last
            # touch) per bank; all other matmuls accumulate with start=False.
            bank_started = [False] * ((sbn + 3) // 4)
            last_bank_window = {}
            for bi in range(sbn):
                last_bank_window[bi // 4] = bi
            for pi, p in enumerate(cfg.pieces_by_si[si]):
                gcol = p["col"]
                bi = p["bi"]
                scol = p["scol"]
                wd = p["width"]
                gt, call = g_tiles[int(cfg.call_of_col[gcol])]
                jin = gcol - call["col_off"]
                s_t = sp.tile([128, 256], F32, tag="s")
                nc.vector.tensor_scalar(
                    s_t[:, :wd], iota_sb[:, :wd],
                    dst_sb[:, scol:scol + 1], w_sb[:, scol:scol + 1],
                    eq, mul,
                )
                nc.tensor.matmul(
                    acc[:, bi * 128: bi * 128 + wd],
                    lhsT=gt[:, jin, :], rhs=s_t[:, :wd],
                    start=not bank_started[bi // 4], stop=False,
                )
                bank_started[bi // 4] = True
            for bi, b in enumerate(sb):
                nc.tensor.matmul(
                    acc[:, bi * 128:(bi + 1) * 128],
                    lhsT=nw2_sb[:], rhs=xt_tile[:, bi * 128:(bi + 1) * 128],
                    start=not bank_started[bi // 4],
                    stop=last_bank_window[bi // 4] == bi,
                )
                bank_started[bi // 4] = True
            for bi, b in enumerate(sb):
                h_t = hp.tile([128, 128], F32, tag="h")
                nc.scalar.activation(h_t[:], acc[:, bi * 128:(bi + 1) * 128],
                                     LRELU, bias=cvec_sb[:, 0:1], scale=1.0,
                                     alpha=SLOPE)
                ps1 = p1p.tile([H, 128], F32, tag="p1")
                nc.tensor.matmul(ps1[:], lhsT=w0t_sb[:], rhs=h_t[:],
                                 start=True, stop=True)
                g_t = hp.tile([128, 128], F32, tag="g2")
                nc.scalar.activation(g_t[:], ps1[:], LRELU,
                                     bias=b0_sb[:, 0:1], scale=1.0, alpha=SLOPE)
                ps2 = p2p.tile([out_c, 128], F32, tag="p2")
                nc.tensor.matmul(ps2[:], lhsT=w1t_sb[:], rhs=g_t[:],
                                 start=True, stop=True)
                nc.scalar.activation(y_sb[:, b * 128:(b + 1) * 128], ps2[:],
                                     LRELU, bias=b1_sb[:, 0:1], scale=1.0,
                                     alpha=SLOPE)

        nc.sync.dma_start(yout[:], y_sb[:])


def declare_tensors(nc, cfg):
    """DRAM tensor declarations; returns (ins, outs) dicts of APs."""
    d = nc.dram_tensor
    ins = dict(
        tbl=d("tbl", [cfg.tbl_rows, H], F32, kind="ExternalInput")[:, :],
        idxs=d("idxs", [128, cfg.totcols * 8], I16, kind="ExternalInput")[:, :],
        dstloc=d("dstloc", [128, cfg.totscols], F32, kind="ExternalInput")[:, :],
        wcol=d("wcol", [128, cfg.totscols], F32, kind="ExternalInput")[:, :],
        xt=d("xt", [H, cfg.pn_pad], F32, kind="ExternalInput")[:, :],
        nw2=d("nw2", [H, H], F32, kind="ExternalInput")[:, :],
        w0t=d("w0t", [H, H], F32, kind="ExternalInput")[:, :],
        w1t=d("w1t", [H, cfg.out_c], F32, kind="ExternalInput")[:, :],
        cvec=d("cvec", [H, 1], F32, kind="ExternalInput")[:, :],
        b0=d("b0", [H, 1], F32, kind="ExternalInput")[:, :],
        b1=d("b1", [cfg.out_c, 1], F32, kind="ExternalInput")[:, :],
        iota=d("iota", [128, 256], F32, kind="ExternalInput")[:, :],
    )
    outs = dict(y=d("y", [cfg.out_c, cfg.pn_pad], F32, kind="ExternalOutput")[:, :])
    return ins, outs


def make_iota():
    return np.ascontiguousarray(
        np.tile(np.arange(256, dtype=np.float32), (128, 1)))


def build_nc(cfg):
    nc = bacc.Bacc("TRN2", target_bir_lowering=False, debug=False,
                   num_devices=cfg.n_cores, num_swdge_queues=3)
    ins, outs = declare_tensors(nc, cfg)
    with tile.TileContext(nc) as tc:
        build_kernel_body(tc, cfg, outs, ins)
    nc.compile()
    return nc


def make_in_maps(cfg, inputs):
    """Full host prep: returns per-core input dicts + the plan cfg."""
    hw = host_weights(inputs)
    edge_index = np.asarray(inputs["edge_index"])
    src = edge_index[0].astype(np.int64)
    dst = edge_index[1].astype(np.int64)
    w = np.asarray(inputs["edge_weight"], np.float32)
    x = np.asarray(inputs["x"], np.float32)

    # per-(core, block, chunk) counts
    pn = cfg.pn
    core = dst // pn
    b = (dst % pn) >> 7
    c = src // cfg.chunk
    flat = (core * cfg.nblk + b) * cfg.nch + c
    counts = np.bincount(flat, minlength=cfg.n_cores * cfg.nblk * cfg.nch)
    counts = counts.reshape(cfg.n_cores, cfg.nblk, cfg.nch)
    plan(cfg, counts)

    iota = make_iota()
    in_maps = []
    for k in range(cfg.n_cores):
        idx2d, dst2d, w2d = host_prep_core(cfg, k, src, dst, w)
        xtk = np.zeros((H, cfg.pn_pad), np.float32)
        xtk[:, :pn] = x[k * pn:(k + 1) * pn].T
        in_maps.append(dict(
            tbl=hw["T"], idxs=idx2d, dstloc=dst2d, wcol=w2d,
            xt=np.ascontiguousarray(xtk),
            nw2=hw["NW2"], w0t=hw["W0T"], w1t=hw["W1T"],
            cvec=hw["cvec"], b0=hw["b0"], b1=hw["b1"], iota=iota,
        ))
    return in_maps


_CACHE = {}
LAST_RESULTS = None


def kernel(**inputs) -> np.ndarray:
    global LAST_RESULTS
    import os
    from concourse.bass_utils import run_bass_kernel_spmd

    cfg = Cfg(N_NODES, N_CORES, tbl_rows=N_NODES)
    in_maps = make_in_maps(cfg, inputs)

    key = ("nc", cfg.totcols, cfg.totscols)
    if key not in _CACHE:
        _CACHE[key] = build_nc(cfg)
    nc = _CACHE[key]

    trace = bool(int(os.environ.get("LINKX_TRACE", "0")))
    res = run_bass_kernel_spmd(nc, in_maps, core_ids=list(range(cfg.n_cores)),
                               trace=trace)
    LAST_RESULTS = res
    out = np.empty((N_NODES, OUT_C), np.float32)
    for k in range(cfg.n_cores):
        yk = res.results[k]["y"]
        out[k * cfg.pn:(k + 1) * cfg.pn] = yk[:, :cfg.pn].T
    return out

